# revision 34
# baseline (speedup 1.0000x reference)
"""Trainium2 Bass kernel for pairwise diagonal-Gaussian KL energies.

energies[b, i] = 0.5 * sum_d [ log(d_id) + (1 + (x_bd - mu_id)^2) / d_id - 1 ]
with d = clip(diag, 1e-6),  x: (4096, 128), mean/diag: (8192, 128).

Sharding: tensor-parallel over codebook rows (n_in) across 8 cores.
Each core gets the full x (host-transposed to [dim, batch], NEGATED, cast
bf16) and a 1024-row shard of mean/diag (host-transposed, packed
[mean C0 | diag C0 | mean rest | diag rest], bf16), and produces the
TRANSPOSED (1024, batch) slab of the output in bf16; the host concatenates
the slabs on axis 0, transposes back to (batch, n_in) and casts f32.

Device math (all in [dim(partition), *] layout):
  inv   = exp(-ln(max(diag,1e-6)))      ScalarE, ONE manual table load
                                        (set 6 holds ln+exp+relu+copy+square)
  minvb = bf16(mean*inv)                DVE mul (x negated on host, so the
                                        minus sign is free)
  invb  = bf16(0.5*inv)                 Pool scalar-mul (so the xx moving
                                        plane is plain x*x)
  xxb   = bf16(x*x)                     DVE muls / Pool mul / SE Squares
  m2i   = minvb*mean (= inv*mean^2)     DVE
  cvp[i]= 0.5*colsum(lg+inv+m2i) - 64   3 accumulating N=1 matmuls per
                                        i-tile + SE Copy(bias=-64)
  per i-tile t: minvb_t.T@x sweep (8 banks, start) then invb_t.T@xx sweep
  (stop); evac DVE (banks 0-2, tensor_scalar_add cvp) + ScalarE (banks
  3-7, Relu with cvp AP bias, exact since KL>=0); two 512 KiB out-DMAs.

Single-shot schedule: input DMAs ordered so the t0/tail-critical codebook
bytes land first (head on sync, diag-C2 behind it, C1 pieces on the two
gpsimd SWDGEs, x streams behind); prep chains run in chunks
(128/256/640 cols) C0 -> C2 -> C1 so both the first matmul (~4us) and the
cvp tail unblock early; cvp batches are injected into the PE stream
during the x-sweeps (their PSUM tile is allocated before the tile's bank
tiles and the matmuls are emitted between banks 6 and 7, keeping the
8-slot PSUM rotation acyclic).

Measured (8x trn2 NC): steady-state pass ~34.0us at BEST unroll=40
(u8 34.9 / u16 34.25 / u32 34.0 / u40 ~33.4-34.0 / u80 33.8 -- the For_i
barrier exposes the evac+DMA tail each iteration), prep-only cost model
10.7us, reported total 44.7us (baseline 52.8us), rel err 5.775e-3 (bf16
operands + bf16 output; identical error to the baseline scheme).
Rejected on measurement: fp8 DoubleRow (S1 residual scheme 2.05e-2 > bar;
S2 full-correction loses the column win to extra planes), N>512 matmuls
(walrus), DVE divide (walrus), gpsimd PSUM reads (walrus), dve_banks=2
(37us), out-DMA ring spreading (no effect).
"""

import numpy as np

N_IN, DIM, BATCH = 8192, 128, 4096
N_CORES = 8
SHARD = N_IN // N_CORES  # 1024 codebook rows per core
PD_THR = 1e-6
IT = SHARD // 128  # 8 i-tiles per core
NB = BATCH // 512  # 8 batch blocks per i-tile

_BUILD_CACHE = {}

# codebook-column chunks for the prep chains
CHUNKS = [(0, 128), (128, 384), (384, 1024)]
# which xx blocks each engine produces ('v' = DVE STT, 's' = ScalarE Square)
XXB_ENG = ["v", "v", "v", "v", "p", "s", "s", "s"]
# cvp batches: i-tile -> cvp columns computed during that tile's x-sweep.
# Each batch's PSUM tile is allocated BEFORE the tile's bank tiles and its
# matmuls are emitted between x-sweep banks 6 and 7, which keeps the 8-slot
# PSUM rotation acyclic.
CVP_HOOKS = {0: (0,), 1: (1, 2), 2: (3, 4, 5), 3: (6, 7)}


def build(
    repeat=1,
    psum_bufs=8,
    out_bufs=3,
    skip_mm=False,
    skip_evac=False,
    skip_out_dma=False,
    out_dtype="bf16",
    unroll=1,
    dve_banks=3,
    split_out=True,
    dma_plan="A",
):
    """Build + compile the single-core SPMD program. Cached per config."""
    key = (
        repeat, psum_bufs, out_bufs, skip_mm, skip_evac, skip_out_dma,
        out_dtype, unroll, dve_banks, split_out, dma_plan,
    )
    if key in _BUILD_CACHE:
        return _BUILD_CACHE[key]

    import concourse.bass as bass
    import concourse.bacc as bacc
    import concourse.tile as tile
    import concourse.mybir as mybir

    f32 = mybir.dt.float32
    bf16 = mybir.dt.bfloat16
    AF = mybir.ActivationFunctionType

    nc = bacc.Bacc("TRN2", target_bir_lowering=False, debug=False)

    odt = bf16 if out_dtype == "bf16" else f32
    xb_d = nc.dram_tensor("xb", [DIM, BATCH], bf16, kind="ExternalInput")
    md_d = nc.dram_tensor("mdt", [DIM, 2 * SHARD], bf16, kind="ExternalInput")
    out_d = nc.dram_tensor("out", [SHARD, BATCH], odt, kind="ExternalOutput")
    out_ap = out_d.ap()
    md_ap = md_d.ap()
    xb_ap = xb_d.ap()

    with tile.TileContext(nc) as tc:
        with (
            tc.tile_pool(name="persist", bufs=1) as pp,
            tc.tile_pool(
                name="psum", bufs=psum_bufs, space=bass.MemorySpace.PSUM
            ) as psm,
            tc.tile_pool(name="outs", bufs=out_bufs) as osp,
        ):
            prep = pp
            # one activation-table load covering ln/exp/relu/square (set 6)
            nc.scalar.add_instruction(
                mybir.InstLoadActFuncSet(
                    name=nc.scalar.bass.get_next_instruction_name(),
                    ins=[],
                    outs=[],
                    act_func_set_id=6,
                )
            )

            # host layout: [mean C0 | diag C0 | mean rest | diag rest]
            md = prep.tile([DIM, 2 * SHARD], bf16)
            HW = CHUNKS[0][1]  # head width (cols of t0 chunk)
            REST = SHARD - HW

            def mcol(sl):
                if sl.stop <= HW:
                    return md[:, sl.start : sl.stop]
                return md[:, 2 * HW + sl.start - HW : 2 * HW + sl.stop - HW]

            def dcol(sl):
                if sl.stop <= HW:
                    return md[:, HW + sl.start : HW + sl.stop]
                return md[
                    :,
                    2 * HW + REST + sl.start - HW : 2 * HW + REST + sl.stop - HW,
                ]

            xb = pp.tile([DIM, BATCH], bf16)

            # ---- input DMAs: t0-critical pieces first ----
            # pool FIFO is roughly issue-order; keep the tiny t0 codebook
            # pieces and x[0:1024] at the head of the queue
            # dma_starts stay OFF the scalar ring: they would serialize in
            # front of the Ln/Exp chain on the Activation sequencer
            if dma_plan == "A":
                # head = [mean C0 | diag C0]; then diag C1 + mean C1 on the
                # gpsimd ring (gates the Ln chain), diag C2 + mean C2 on
                # sync behind x0 (gates the prep tail), x tail last
                DGR = 2 * HW + REST  # diag-rest base col
                C2o = CHUNKS[2][0] - HW  # rest-relative offset of the acc chunks
                nc.sync.dma_start(md[:, 0 : 2 * HW], md_ap[:, 0 : 2 * HW])
                # diag C2 right behind the head on sync: it gates the prep
                # tail (clip2 -> Ln2 -> Exp2 -> m2i2 -> cvp); mean C2 after
                # x0; the C1 pieces ride the gpsimd ring (max 2 SWDGEs --
                # each one costs ~1us of Pool engine time)
                nc.sync.dma_start(
                    md[:, DGR + C2o :], md_ap[:, DGR + C2o :]
                )
                nc.gpsimd.dma_start(
                    md[:, DGR : DGR + C2o], md_ap[:, DGR : DGR + C2o]
                )
                nc.gpsimd.dma_start(
                    md[:, 2 * HW : 2 * HW + C2o],
                    md_ap[:, 2 * HW : 2 * HW + C2o],
                )
                nc.sync.dma_start(
                    md[:, 2 * HW + C2o : DGR],
                    md_ap[:, 2 * HW + C2o : DGR],
                )
                nc.sync.dma_start(xb[:, 0:1024], xb_ap[:, 0:1024])
                nc.sync.dma_start(xb[:, 1024:2560], xb_ap[:, 1024:2560])
                nc.sync.dma_start(xb[:, 2560:4096], xb_ap[:, 2560:4096])
            else:
                nc.sync.dma_start(md[:, 0 : 2 * HW], md_ap[:, 0 : 2 * HW])
                nc.sync.dma_start(xb[:, 0:1024], xb_ap[:, 0:1024])
                nc.gpsimd.dma_start(md[:, 2 * HW :], md_ap[:, 2 * HW :])
                nc.sync.dma_start(xb[:, 1024:2560], xb_ap[:, 1024:2560])
                nc.sync.dma_start(xb[:, 2560:4096], xb_ap[:, 2560:4096])

            half_col = pp.tile([DIM, 1], f32)
            nc.vector.memset(half_col[:], 0.5)

            dc = prep.tile([DIM, SHARD], f32)
            lg = prep.tile([DIM, SHARD], f32)
            inv = prep.tile([DIM, SHARD], f32)
            m2i = prep.tile([DIM, SHARD], f32)
            cvp = pp.tile([DIM, IT], f32)
            invb = pp.tile([DIM, SHARD], bf16)
            minvb = pp.tile([DIM, SHARD], bf16)
            xxb = pp.tile([DIM, BATCH], bf16)

            def _sl(c):
                lo, hi = CHUNKS[c]
                return slice(lo, hi)

            # granular prep emitters (engine in parens)
            def e_clip(c):  # DVE
                sl = _sl(c)
                return nc.vector.tensor_scalar_max(dc[:, sl], dcol(sl), PD_THR)

            def e_ln(c):  # SE
                sl = _sl(c)
                nc.scalar.activation(lg[:, sl], dc[:, sl], AF.Ln, bias=0.0)

            def e_exp(c):  # SE
                sl = _sl(c)
                nc.scalar.activation(
                    inv[:, sl], lg[:, sl], AF.Exp, bias=0.0, scale=-1.0
                )

            def e_minvb(c):  # DVE: +mean*inv (host negated x instead)
                sl = _sl(c)
                nc.vector.tensor_mul(minvb[:, sl], mcol(sl), inv[:, sl])

            def e_m2i(c):  # DVE: m2i = minvb*mean = inv*mean^2 (bf16 minvb)
                sl = _sl(c)
                return nc.vector.tensor_mul(m2i[:, sl], minvb[:, sl], mcol(sl))

            def e_invb(c):  # Pool: invb = 0.5*inv (xx plane is plain x*x)
                sl = _sl(c)
                nc.gpsimd.tensor_scalar_mul(invb[:, sl], inv[:, sl], 0.5)

            def xxb_blk(b):
                # deprioritized: the scheduler must not hoist xx prep into
                # the Ln/Exp/m2i critical chain
                bs = slice(b * 512, (b + 1) * 512)
                if XXB_ENG[b] == "v":
                    i = nc.vector.tensor_mul(xxb[:, bs], xb[:, bs], xb[:, bs])
                elif XXB_ENG[b] == "p":
                    i = nc.gpsimd.tensor_mul(xxb[:, bs], xb[:, bs], xb[:, bs])
                else:
                    i = nc.scalar.activation(
                        xxb[:, bs], xb[:, bs], AF.Square, bias=0.0
                    )
                i.bass_priority = 50000 + b

            def cvp_mms(ts, cps):
                # cvp[i] = 0.5*colsum(lg+inv+m2i)[i] - 64 per i-tile t in
                # ts via 3 accumulating N=1 matmuls (PE is idle during
                # prep); evacuated on ScalarE with the -64 as bias
                for j, t in enumerate(ts):
                    isl = slice(t * 128, (t + 1) * 128)
                    nc.tensor.matmul(
                        cps[:, j : j + 1], lg[:, isl], half_col[:],
                        start=True, stop=False,
                    )
                    nc.tensor.matmul(
                        cps[:, j : j + 1], inv[:, isl], half_col[:],
                        start=False, stop=False,
                    )
                    nc.tensor.matmul(
                        cps[:, j : j + 1], m2i[:, isl], half_col[:],
                        start=False, stop=True,
                    )
                i = nc.scalar.activation(
                    cvp[:, ts[0] : ts[0] + len(ts)], cps[:],
                    AF.Copy, bias=-64.0,
                )
                i.bass_priority = 3

            def main_tile(t, hooks=True):
                isl = slice(t * 128, (t + 1) * 128)
                hk = CVP_HOOKS.get(t) if hooks else None
                cps = (
                    psm.tile([DIM, len(hk)], f32, tag="ps", name="cps")
                    if hk
                    else None
                )
                pss = []
                if not skip_mm:
                    # sweep 1: minv.T @ x (start); the cvp batch for this
                    # tile slots in before the last bank
                    for b in range(NB):
                        bs = slice(b * 512, (b + 1) * 512)
                        ps = psm.tile([128, 512], f32, tag="ps")
                        pss.append(ps)
                        if b == NB - 1 and hk:
                            cvp_mms(hk, cps)
                        nc.tensor.matmul(
                            ps[:], minvb[:, isl], xb[:, bs],
                            start=True, stop=False,
                        )
                elif hk:
                    cvp_mms(hk, cps)
                if not skip_mm:
                    # sweep 2: inv.T @ xx (stop)
                    for b in range(NB):
                        bs = slice(b * 512, (b + 1) * 512)
                        nc.tensor.matmul(
                            pss[b][:], invb[:, isl], xxb[:, bs],
                            start=False, stop=True,
                        )
                ob = osp.tile([128, BATCH], odt, tag="ob", name="ob")
                if not skip_evac:
                    for b in range(NB):
                        bs = slice(b * 512, (b + 1) * 512)
                        src = pss[b][:] if not skip_mm else xb[:, bs]
                        if b < dve_banks:
                            nc.vector.tensor_scalar_add(
                                ob[:, bs], src, cvp[:, t : t + 1]
                            )
                        else:
                            # energies are KL >= 0: Relu is an exact copy
                            # and accepts the per-partition AP bias
                            nc.scalar.activation(
                                ob[:, bs], src, AF.Relu,
                                bias=cvp[:, t : t + 1],
                            )
                if not skip_out_dma:
                    osl = slice(t * 128, (t + 1) * 128)
                    if split_out:
                        nc.sync.dma_start(
                            out_ap[osl, 0:2048], ob[:, 0:2048]
                        )
                        nc.sync.dma_start(
                            out_ap[osl, 2048:4096], ob[:, 2048:4096]
                        )
                    else:
                        nc.sync.dma_start(out_ap[osl, :], ob[:])

            # ---- prep emission: C0 first (ramp), then the C2 tail
            # chain (cvp-critical), then C1; xx blocks fill the gaps ----
            e_clip(0)
            e_ln(0)
            e_exp(0)
            i = e_clip(2)
            i.bass_priority = 1
            e_ln(2)
            e_exp(2)
            e_minvb(0)
            e_m2i(0)
            e_clip(1)
            e_ln(1)
            e_exp(1)
            e_minvb(2)
            i = e_m2i(2)
            i.bass_priority = 2
            e_minvb(1)
            e_m2i(1)
            e_invb(0)
            e_invb(2)
            e_invb(1)
            for b in range(NB):
                xxb_blk(b)

            if repeat > 1:
                for ts in ((0, 1), (2,), (3, 4, 5, 6, 7)):
                    cps = psm.tile([DIM, len(ts)], f32, tag="ps", name="cps")
                    cvp_mms(ts, cps)
                assert repeat % unroll == 0
                with tc.For_i(0, repeat // unroll, 1):
                    for _ in range(unroll):
                        for t in range(IT):
                            main_tile(t, hooks=False)
            else:
                for t in range(IT):
                    main_tile(t)

    nc.compile()
    _BUILD_CACHE[key] = nc
    return nc


def make_in_maps(x, mean, diag):
    import ml_dtypes

    # x is negated on the host: the x-GEMM stationary becomes +mean*inv
    # (plain mul, no STT) and x*x / Square are sign-invariant
    xb = np.ascontiguousarray((-np.asarray(x)).T.astype(ml_dtypes.bfloat16))
    hw = CHUNKS[0][1]
    in_maps = []
    for c in range(N_CORES):
        sl = slice(c * SHARD, (c + 1) * SHARD)
        mT = np.asarray(mean)[sl].T
        dT = np.asarray(diag)[sl].T
        # [mean C0 | diag C0 | mean rest | diag rest]
        md = np.concatenate(
            [mT[:, :hw], dT[:, :hw], mT[:, hw:], dT[:, hw:]], axis=1
        ).astype(ml_dtypes.bfloat16)
        in_maps.append({"xb": xb, "mdt": np.ascontiguousarray(md)})
    return in_maps


# best measured config, used by kernel() and by test.py's timing builds
BEST = {"unroll": 40}


def kernel(x, mean, diag):
    from concourse.bass_utils import run_bass_kernel_spmd

    nc = build(repeat=1, **BEST)
    in_maps = make_in_maps(x, mean, diag)
    try:
        res = run_bass_kernel_spmd(nc, in_maps, list(range(N_CORES)))
    except Exception:
        # rare transient device error; one retry
        res = run_bass_kernel_spmd(nc, in_maps, list(range(N_CORES)))
    outT = np.concatenate(
        [res.results[c]["out"] for c in range(N_CORES)], axis=0
    ).astype(np.float32)
    return np.ascontiguousarray(outT.T)
